# revision 24
# baseline (speedup 1.0000x reference)
"""CL4KT transformer layer kernel for Trainium2 (8 NeuronCores, data-parallel).

Layout strategy:
  - Activations kept feature-major ([D, S], feature on partitions) so every GEMM
    uses weights as the stationary operand; activation transposes (PE identity
    matmuls) happen only on q/k/v input, per-head p, and the final output.
  - The distance-decay total_effect term is computed with a DVE prefix scan
    (cumsum) instead of a triangular matmul; sqrt(u) is computed as
    exp(0.5*ln(u)) so the ACT engine stays on one spline table set.
  - All big matmuls run in float32r (4x fp32 PE throughput, ~1e-4 rel err).
  - FFN: W1 streamed once over all batches (h spilled to DRAM scratch), then
    W2 resident with PSUM-accumulated output; LN2 + output transpose last.
  - Causal mask: per-row-chunk free widths shrink all softmax/PE work to the
    allowed region; masked p_attn columns are zero-filled via gpsimd memset.
"""
import sys

if "/opt/trn_rl_repo" not in sys.path:
    sys.path.insert(0, "/opt/trn_rl_repo")

import numpy as np
from contextlib import ExitStack

import bass_rust as _bass_rust
import concourse.bacc as bacc
import concourse.mybir as mybir
import concourse.tile as tile
from concourse.hw_specs import get_activation_tables
from concourse.masks import make_identity
from concourse.bass_utils import run_bass_kernel_spmd

# Activation-table sets that precede natural_log_exp_and_others in
# act_info.json and contain overlapping funcs (exp/square/copy/...). The
# first-match chooser in insert_act_table_loads would otherwise alternate
# table loads between exp_and_others and natural_log_exp_and_others on
# every Ln <-> Exp transition (1.3us per reload). Presenting these sets as
# empty makes every phase-A func resolve to the one set that has them all,
# while keeping set ids aligned with walrus' act_info.json.
_DEMOTE_ACT_SETS = {"exp_and_others", "softplus_and_others",
                    "sigmoid_and_others", "sqrt_and_others", "small"}


class _Bacc(bacc.Bacc):
    def insert_act_table_loads(self):
        has_activation = any(
            isinstance(i, mybir.InstActivation)
            for b in self.main_func.blocks
            for i in b.instructions
        )
        if not has_activation:
            return
        tables = [
            (name, set() if name in _DEMOTE_ACT_SETS else funcs)
            for name, funcs in get_activation_tables(self.m.arch).items()
        ]
        _bass_rust.insert_act_table_loads(self, tables)

F32 = mybir.dt.float32
F32R = mybir.dt.float32r
AF = mybir.ActivationFunctionType
OP = mybir.AluOpType

B, S, D, H = 64, 512, 1024, 16
DK = D // H          # 64
DFF = 4096
NC_ = 8              # cores
BSH = B // NC_       # batches per core
KC = D // 128        # 8
FC = DFF // 128      # 32
SC = S // 128        # 4
LN_EPS = 1e-5


def _chunk_w(c, mask):
    if mask <= 0:
        return S
    return int(min(S, 128 * (c + 1) + mask - 1))


def build_nc(mask, gam_neg, apply_ln1_affine, apply_ln2_affine,
             phases="A123", nbatch=BSH, nheads=H, apply_bv=False,
             apply_bk=True):
    nc = _Bacc("TRN2", target_bir_lowering=False, debug=False,
                num_devices=NC_)

    dp = lambda n, sh: nc.declare_dram_parameter(n, sh, F32, isOutput=False)
    q_in = dp("q", [BSH, S, D])
    k_in = dp("k", [BSH, S, D])
    v_in = dp("v", [BSH, S, D])
    Wk_in = dp("Wk", [D, D])
    Wv_in = dp("Wv", [D, D])
    Wo_in = dp("Wo", [D, D])
    W1_in = dp("W1p", [FC, 128, D])      # host-packed [fc, d_part, kc*128+f]
    W2_in = dp("W2", [DFF, D])
    bk_in = dp("bkc", [128, KC])
    bo_in = dp("boc", [128, KC])
    b1_in = dp("b1c", [128, FC])
    b2_in = dp("b2c", [128, KC])
    bv_in = dp("bvr", [1, D])
    allow_in = nc.declare_dram_parameter("allow_s", [S, S],
                                         mybir.dt.float16, isOutput=False)
    posn_in = nc.declare_dram_parameter("pos_neg", [S, S],
                                        mybir.dt.float16, isOutput=False)
    ln1w_in = dp("ln1wc", [128, KC])
    ln1b_in = dp("ln1bc", [128, KC])
    ln2w_in = dp("ln2wb", [1, D])
    ln2b_in = dp("ln2bb", [1, D])

    x_out = nc.declare_dram_parameter("x_out", [BSH, S, D], F32, isOutput=True)
    p_out = nc.declare_dram_parameter("p_out", [BSH, H, S, S], F32,
                                      isOutput=True)

    x1_scr = nc.dram_tensor("x1_scr", [BSH, KC, 128, S], F32R)
    xq_scr = nc.dram_tensor("xq_scr", [KC, 128, S], F32R)
    h_scr = nc.dram_tensor("h_scr", [BSH, FC, 128, S], F32R)
    z2_scr = nc.dram_tensor("z2_scr", [BSH, KC, 128, S], F32)

    with ExitStack() as top:
        tc = top.enter_context(tile.TileContext(nc))
        const = top.enter_context(tc.tile_pool(name="const", bufs=1))

        ident = const.tile([128, 128], F32, tag="ident")
        make_identity(nc, ident)
        allow_t = const.tile([128, SC, S], mybir.dt.float16, tag="allow")
        posn_t = const.tile([128, SC, S], mybir.dt.float16, tag="posn")
        for c in range(SC):
            nc.sync.dma_start(out=allow_t[:, c, :],
                              in_=allow_in[128 * c:128 * (c + 1), :])
            nc.sync.dma_start(out=posn_t[:, c, :],
                              in_=posn_in[128 * c:128 * (c + 1), :])
        bk_c = const.tile([128, KC], F32, tag="bkc")
        bo_c = const.tile([128, KC], F32, tag="boc")
        b1_c = const.tile([128, FC], F32, tag="b1c")
        b2_c = const.tile([128, KC], F32, tag="b2c")
        nc.sync.dma_start(out=bk_c, in_=bk_in[:, :])
        nc.sync.dma_start(out=bo_c, in_=bo_in[:, :])
        nc.sync.dma_start(out=b1_c, in_=b1_in[:, :])
        nc.sync.dma_start(out=b2_c, in_=b2_in[:, :])
        ln1w_c = const.tile([128, KC], F32, tag="ln1wc")
        ln1b_c = const.tile([128, KC], F32, tag="ln1bc")
        nc.sync.dma_start(out=ln1w_c, in_=ln1w_in[:, :])
        nc.sync.dma_start(out=ln1b_c, in_=ln1b_in[:, :])
        bv_r = None
        if apply_bv:
            bv_r = const.tile([1, D], F32R, tag="bvr")
        ones_col_f = const.tile([128, 1], F32, tag="onescf")
        nc.vector.memset(ones_col_f, 1.0)
        ones_col = const.tile([128, 1], F32R, tag="onesc")
        nc.vector.tensor_copy(ones_col, ones_col_f)
        ones_row_f = const.tile([1, 128], F32, tag="onesrf")
        nc.vector.memset(ones_row_f, 1.0)
        ones_row = const.tile([1, 128], F32R, tag="onesr")
        nc.vector.tensor_copy(ones_row, ones_row_f)
        eps_1 = const.tile([1, 1], F32, tag="eps1")
        nc.vector.memset(eps_1, LN_EPS)
        eps_128 = const.tile([128, 1], F32, tag="eps128")
        nc.vector.memset(eps_128, LN_EPS)

        # ---------------- Phase A: attention + LN1 (per batch) -------------
        with ExitStack() as pa:
            stage = pa.enter_context(tc.tile_pool(name="stage", bufs=2))
            wsl = pa.enter_context(tc.tile_pool(name="wsl", bufs=1))
            actv = pa.enter_context(tc.tile_pool(name="actv", bufs=1))
            smx = pa.enter_context(tc.tile_pool(name="smx", bufs=3))
            smt = pa.enter_context(tc.tile_pool(name="smt", bufs=3))
            lnp = pa.enter_context(tc.tile_pool(name="lnp", bufs=1))
            ptp = pa.enter_context(tc.tile_pool(name="ptp", bufs=1))
            php = pa.enter_context(tc.tile_pool(name="php", bufs=2))
            ps_big = pa.enter_context(
                tc.tile_pool(name="ps_big", bufs=5, space="PSUM"))
            ps_tp = pa.enter_context(
                tc.tile_pool(name="ps_tp", bufs=2, space="PSUM"))
            ps_cx = pa.enter_context(
                tc.tile_pool(name="ps_cx", bufs=1, space="PSUM"))

            if "A" in phases and apply_bv:
                bv_f = stage.tile([128, 512], F32, tag="wst")
                nc.sync.dma_start(out=bv_f[0:1, :512], in_=bv_in[:, :512])
                nc.sync.dma_start(out=bv_f[1:2, :512], in_=bv_in[:, 512:])
                nc.vector.tensor_copy(bv_r[:, :512], bv_f[0:1, :512])
                nc.vector.tensor_copy(bv_r[:, 512:], bv_f[1:2, :512])

            def wk_half(w_dram, half):
                """[128, KC, 512] f32r: columns 512*half.. of a [D,D] weight."""
                wr = wsl.tile([128, KC, 512], F32R, tag="wslr")
                for kc in range(KC):
                    wst = stage.tile([128, 512], F32, tag="wst")
                    nc.sync.dma_start(
                        out=wst,
                        in_=w_dram[128 * kc:128 * (kc + 1),
                                   512 * half:512 * (half + 1)])
                    nc.gpsimd.tensor_copy(wr[:, kc, :], wst)
                return wr

            for b in range(nbatch if "A" in phases else 0):
                # ---- q/k/v natural -> feature-major via PE transpose ----
                xq = actv.tile([128, KC, S], F32R, tag="xqcx")
                xk = actv.tile([128, KC, S], F32R, tag="xkv")
                for (src, dst) in ((q_in, xq), (k_in, xk)):
                    for sc in range(SC):
                        nat = stage.tile([128, D], F32, tag="nat")
                        nc.sync.dma_start(
                            out=nat, in_=src[b, 128 * sc:128 * (sc + 1), :])
                        for dcc in range(KC):
                            tp = ps_tp.tile([128, 128], F32, tag="tps")
                            nc.tensor.transpose(
                                tp, nat[:, 128 * dcc:128 * (dcc + 1)], ident)
                            nc.scalar.copy(
                                dst[:, dcc, 128 * sc:128 * (sc + 1)], tp)

                # ---- Q/K projections (per Wk column-slice) ----
                qT = actv.tile([128, KC, S], F32R, tag="qT")
                kT = actv.tile([128, KC, S], F32R, tag="kT")
                for half in range(2):
                    wr = wk_half(Wk_in, half)
                    for mq in range(4):
                        mc = 4 * half + mq
                        for (xin, dst) in ((xq, qT), (xk, kT)):
                            pp = ps_big.tile([128, 512], F32, tag="big")
                            for kc in range(KC):
                                nc.tensor.matmul(
                                    pp, wr[:, kc, 128 * mq:128 * (mq + 1)],
                                    xin[:, kc, :],
                                    start=(kc == 0), stop=(kc == KC - 1))
                            if apply_bk:
                                nc.scalar.activation(dst[:, mc, :], pp,
                                                     AF.Identity,
                                                     bias=bk_c[:, mc:mc + 1])
                            else:
                                nc.vector.tensor_copy(dst[:, mc, :], pp)
                # spill queryT for the residual; frees the xq slot for ctxT
                nc.sync.dma_start(
                    out=xq_scr.rearrange("kc p s -> p kc s"), in_=xq)

                # ---- V (natural layout) ----
                xv = actv.tile([128, KC, S], F32R, tag="xkv")
                for sc in range(SC):
                    nat = stage.tile([128, D], F32, tag="nat")
                    nc.sync.dma_start(
                        out=nat, in_=v_in[b, 128 * sc:128 * (sc + 1), :])
                    for dcc in range(KC):
                        tp = ps_tp.tile([128, 128], F32, tag="tps")
                        nc.tensor.transpose(
                            tp, nat[:, 128 * dcc:128 * (dcc + 1)], ident)
                        nc.scalar.copy(xv[:, dcc, 128 * sc:128 * (sc + 1)],
                                       tp)
                vN = actv.tile([128, SC, D], F32R, tag="vN")
                for nn in range(2):
                    wvh = wk_half(Wv_in, nn)
                    for sc in range(SC):
                        pp = ps_big.tile([128, 512], F32, tag="big")
                        for kc in range(KC):
                            nc.tensor.matmul(
                                pp, xv[:, kc, 128 * sc:128 * (sc + 1)],
                                wvh[:, kc, :], start=(kc == 0),
                                stop=(not apply_bv and kc == KC - 1))
                        if apply_bv:
                            nc.tensor.matmul(
                                pp, ones_row,
                                bv_r[:, 512 * nn:512 * (nn + 1)],
                                start=False, stop=True)
                        nc.vector.tensor_copy(vN[:, sc, 512 * nn:512 * (nn + 1)], pp)

                # ---- heads ----
                ctxT = actv.tile([128, KC, S], F32R, tag="xqcx")
                for h in range(nheads):
                    hp = 64 * (h % 2)
                    hc = h // 2
                    pTs = [ptp.tile([128, S - 128 * jb], F32R, tag=f"pT{jb}",
                                    name=f"pT{jb}") for jb in range(SC)]
                    ph = php.tile([128, SC, S], F32, tag="ph")
                    ctx_ps = ps_cx.tile([64, 512], F32, tag="ctxps")
                    for c in range(SC):
                        W = _chunk_w(c, mask)
                        scp = ps_big.tile([128, 512], F32, tag="big")
                        nc.tensor.matmul(
                            scp[:, :W],
                            qT[hp:hp + 64, hc, 128 * c:128 * (c + 1)],
                            kT[hp:hp + 64, hc, :W], start=True, stop=True)
                        e1 = smx.tile([128, S], F32, tag="smA")
                        nc.scalar.activation(e1[:, :W], scp[:, :W], AF.Exp,
                                             scale=0.125)
                        # reverse masked inclusive scan:
                        # buf[j] = sum_{j'>=j, allowed} e1[j'];  buf[W] = 0
                        buf = smx.tile([128, S + 1], F32, tag="smB")
                        nc.gpsimd.memset(buf[:, W:W + 1], 0.0)
                        nc.vector.tensor_tensor_scan(
                            buf[:, 0:W][:, ::-1], e1[:, :W][:, ::-1],
                            allow_t[:, c, :W][:, ::-1], 0.0, OP.add, OP.mult)
                        rz1 = smt.tile([128, 1], F32, tag="rz1")
                        nc.vector.reciprocal(rz1, buf[:, 0:1])
                        # u = revcum_strict * pos   (the /Z1 goes into Ln scale)
                        u = smx.tile([128, S], F32, tag="smC")
                        nc.vector.tensor_mul(u[:, :W], buf[:, 1:W + 1],
                                             posn_t[:, c, :W])
                        lnu = smx.tile([128, S], F32, tag="smD")
                        nc.scalar.activation(lnu[:, :W], u[:, :W], AF.Ln,
                                             scale=rz1)
                        sqv = smx.tile([128, S], F32, tag="smE")
                        nc.scalar.activation(sqv[:, :W], lnu[:, :W], AF.Exp,
                                             scale=0.5)
                        te = smx.tile([128, S], F32, tag="smF")
                        nc.scalar.activation(te[:, :W], sqv[:, :W], AF.Exp,
                                             scale=gam_neg[h])
                        # s2 = max(te, 1e-5) * scores   (the /8 goes into e2)
                        s2 = smx.tile([128, S], F32, tag="smG")
                        nc.vector.scalar_tensor_tensor(
                            out=s2[:, :W], in0=te[:, :W], scalar=1e-5,
                            in1=scp[:, :W], op0=OP.max, op1=OP.mult)
                        e2 = smx.tile([128, S], F32, tag="smH")
                        nc.scalar.activation(e2[:, :W], s2[:, :W], AF.Exp,
                                             scale=0.125)
                        pm = smx.tile([128, S], F32, tag="smI")
                        z2t = smt.tile([128, 1], F32, tag="z2t")
                        nc.vector.scalar_tensor_tensor(
                            out=pm[:, :W], in0=e2[:, :W], scalar=1.0,
                            in1=allow_t[:, c, :W], op0=OP.mult, op1=OP.mult,
                            accum_out=z2t)
                        rz2 = smt.tile([128, 1], F32, tag="rz2")
                        nc.vector.reciprocal(rz2, z2t)
                        p = ph[:, c, :]
                        nc.gpsimd.tensor_scalar_mul(p[:, :W], pm[:, :W], rz2)
                        if W < S:
                            nc.gpsimd.memset(p[:, W:], 0.0)
                        for jb in range(min(c + 1, SC)):
                            tp = ps_tp.tile([128, 128], F32, tag="tps")
                            nc.tensor.transpose(
                                tp, p[:, 128 * jb:128 * (jb + 1)], ident)
                            nc.vector.tensor_copy(
                                pTs[jb][:, 128 * (c - jb):
                                        128 * (c - jb + 1)], tp)
                    nc.sync.dma_start(
                        out=p_out[b, h].rearrange("(c p) j -> p c j", p=128),
                        in_=ph)
                    for jb in range(SC):
                        nc.tensor.matmul(
                            ctx_ps[:, 128 * jb:],
                            vN[:, jb, DK * h:DK * (h + 1)],
                            pTs[jb],
                            start=(jb == 0), stop=(jb == SC - 1))
                    nc.scalar.copy(ctxT[hp:hp + 64, hc, :], ctx_ps)

                # ---- out projection + residual ----
                z1T = actv.tile([128, KC, S], F32R, tag="qT")
                for half in range(2):
                    wr = wk_half(Wo_in, half)
                    for mq in range(4):
                        mc = 4 * half + mq
                        pp = ps_big.tile([128, 512], F32, tag="big")
                        for kc in range(KC):
                            nc.tensor.matmul(
                                pp, wr[:, kc, 128 * mq:128 * (mq + 1)],
                                ctxT[:, kc, :],
                                start=(kc == 0), stop=(kc == KC - 1))
                        xqs = stage.tile([128, 512], F32R, tag="wst")
                        nc.sync.dma_start(out=xqs, in_=xq_scr[mc])
                        nc.vector.scalar_tensor_tensor(
                            out=z1T[:, mc, :], in0=pp,
                            scalar=bo_c[:, mc:mc + 1],
                            in1=xqs.bitcast(F32), op0=OP.add, op1=OP.add)

                # ---- LN1 (feature-major, stats via ones-matmul) ----
                sz_ps = ps_big.tile([128, 512], F32, tag="big")
                sq_ps = ps_big.tile([128, 512], F32, tag="big")
                for kc in range(KC):
                    sq = smx.tile([128, S], F32R, tag="smA")
                    nc.scalar.activation(sq, z1T[:, kc, :].bitcast(F32),
                                         AF.Square)
                    nc.tensor.matmul(sz_ps[0:1, :], ones_col, z1T[:, kc, :],
                                     start=(kc == 0), stop=(kc == KC - 1))
                    nc.tensor.matmul(sq_ps[0:1, :], ones_col, sq,
                                     start=(kc == 0), stop=(kc == KC - 1))
                mu = lnp.tile([1, S], F32R, tag="mu")
                nc.vector.tensor_scalar(out=mu, in0=sz_ps[0:1, :],
                                        scalar1=1.0 / D, scalar2=None,
                                        op0=OP.mult)
                ex2 = lnp.tile([1, S], F32, tag="ex2")
                nc.vector.tensor_scalar(out=ex2, in0=sq_ps[0:1, :],
                                        scalar1=1.0 / D, scalar2=None,
                                        op0=OP.mult)
                mm_t = lnp.tile([1, S], F32, tag="lt1")
                nc.vector.tensor_mul(mm_t, mu.bitcast(F32), mu.bitcast(F32))
                nc.vector.tensor_sub(ex2, ex2, mm_t)
                nc.scalar.activation(mm_t, ex2, AF.Ln, bias=eps_1)
                rstd = lnp.tile([1, S], F32R, tag="rstd")
                nc.scalar.activation(rstd, mm_t, AF.Exp, scale=-0.5)
                mu_bc_ps = ps_big.tile([128, 512], F32, tag="big")
                nc.tensor.matmul(mu_bc_ps, ones_row, mu, start=True, stop=True)
                mu_bc = lnp.tile([128, S], F32, tag="mubc")
                nc.scalar.copy(mu_bc, mu_bc_ps)
                rs_bc_ps = ps_big.tile([128, 512], F32, tag="big")
                nc.tensor.matmul(rs_bc_ps, ones_row, rstd, start=True,
                                 stop=True)
                rs_bc = lnp.tile([128, S], F32, tag="rsbc")
                nc.scalar.copy(rs_bc, rs_bc_ps)
                for kc in range(KC):
                    t0 = smx.tile([128, S], F32, tag="smA")
                    nc.vector.tensor_sub(t0, z1T[:, kc, :].bitcast(F32), mu_bc)
                    x1c = stage.tile([128, S], F32R, tag="wst")
                    if apply_ln1_affine:
                        t1 = smx.tile([128, S], F32, tag="smB")
                        nc.vector.tensor_mul(t1, t0, rs_bc)
                        nc.vector.tensor_scalar(
                            out=x1c, in0=t1,
                            scalar1=ln1w_c[:, kc:kc + 1],
                            scalar2=ln1b_c[:, kc:kc + 1],
                            op0=OP.mult, op1=OP.add)
                    else:
                        nc.vector.tensor_mul(x1c, t0, rs_bc)
                    nc.sync.dma_start(out=x1_scr[b, kc], in_=x1c)

        # ---------------- Phase F1: h = gelu(x1 @ W1 + b1) ----------------
        with ExitStack() as pf:
            x1p = pf.enter_context(tc.tile_pool(name="x1p", bufs=1))
            w1p = pf.enter_context(tc.tile_pool(name="w1p", bufs=2))
            hfp = pf.enter_context(tc.tile_pool(name="hfp", bufs=3))
            psf = pf.enter_context(tc.tile_pool(name="psf", bufs=4, space="PSUM"))

            x1_all = x1p.tile([128, BSH, KC, S], F32R, tag="x1all")
            for b in range(nbatch if "1" in phases else 0):
                nc.sync.dma_start(
                    out=x1_all[:, b],
                    in_=x1_scr[b].rearrange("kc p s -> p kc s"))
            for fc in range(FC if "1" in phases else 0):
                w1f = w1p.tile([128, D], F32, tag="w1f")
                nc.sync.dma_start(out=w1f, in_=W1_in[fc])
                w1r = w1p.tile([128, KC, 128], F32R, tag="w1r")
                nc.gpsimd.tensor_copy(
                    w1r, w1f.rearrange("p (kc f) -> p kc f", kc=KC))
                for b0 in range(0, nbatch, 2):
                    nb2 = min(2, nbatch - b0)
                    hf = hfp.tile([128, 2, S], F32R, tag="hf")
                    for bi in range(nb2):
                        pp = psf.tile([128, 512], F32, tag="ppf")
                        for kc in range(KC):
                            nc.tensor.matmul(pp, w1r[:, kc, :],
                                             x1_all[:, b0 + bi, kc, :],
                                             start=(kc == 0),
                                             stop=(kc == KC - 1))
                        nc.scalar.activation(hf[:, bi, :], pp, AF.Gelu,
                                             bias=b1_c[:, fc:fc + 1])
                    nc.sync.dma_start(
                        out=h_scr[b0:b0 + nb2, fc].rearrange("b p s -> p b s"),
                        in_=hf[:, :nb2, :])

        # ---------------- Phase F2: z2 = x1 + h @ W2 + b2 ----------------
        with ExitStack() as pf:
            w2p = pf.enter_context(tc.tile_pool(name="w2p", bufs=1))
            w2s = pf.enter_context(tc.tile_pool(name="w2s", bufs=2))
            x1bp = pf.enter_context(tc.tile_pool(name="x1bp", bufs=2))
            hsp = pf.enter_context(tc.tile_pool(name="hsp", bufs=3))
            z2p = pf.enter_context(tc.tile_pool(name="z2p", bufs=2))
            psff = pf.enter_context(tc.tile_pool(name="psff", bufs=1, space="PSUM"))

            w2r = w2p.tile([128, FC, D], F32R, tag="w2r")
            for fc in range(FC if "2" in phases else 0):
                wst = w2s.tile([128, D], F32, tag="w2st")
                nc.sync.dma_start(out=wst,
                                  in_=W2_in[128 * fc:128 * (fc + 1), :])
                nc.gpsimd.tensor_copy(w2r[:, fc, :], wst)
            for b in range(nbatch if "2" in phases else 0):
                x1b = x1bp.tile([128, KC, S], F32R, tag="x1b")
                nc.sync.dma_start(
                    out=x1b, in_=x1_scr[b].rearrange("kc p s -> p kc s"))
                ffpA = psff.tile([128, 4, 512], F32, tag="ffpA")
                ffpB = psff.tile([128, 4, 512], F32, tag="ffpB")
                halves = (ffpA, ffpB)
                for fc0 in range(0, FC, 2):
                    hf = hsp.tile([128, 2, S], F32R, tag="hfs")
                    nc.sync.dma_start(
                        out=hf, in_=h_scr[b, fc0:fc0 + 2]
                        .rearrange("f p s -> p f s"))
                    for fi in range(2):
                        fc = fc0 + fi
                        for mc in range(KC):
                            nc.tensor.matmul(
                                halves[mc // 4][:, mc % 4, :],
                                w2r[:, fc, 128 * mc:128 * (mc + 1)],
                                hf[:, fi, :], start=(fc == 0),
                                stop=(fc == FC - 1))
                for mc in range(KC):
                    z2 = z2p.tile([128, S], F32, tag="z2")
                    nc.vector.scalar_tensor_tensor(
                        out=z2, in0=halves[mc // 4][:, mc % 4, :],
                        scalar=b2_c[:, mc:mc + 1],
                        in1=x1b[:, mc, :].bitcast(F32), op0=OP.add, op1=OP.add)
                    nc.sync.dma_start(out=z2_scr[b, mc], in_=z2)


        # ---------------- Phase F3: LN2 + transpose + output --------------
        with ExitStack() as pf:
            z2tp = pf.enter_context(tc.tile_pool(name="z2tp", bufs=2))
            natp = pf.enter_context(tc.tile_pool(name="natp", bufs=2))
            st3 = pf.enter_context(tc.tile_pool(name="st3", bufs=2))
            ps3 = pf.enter_context(tc.tile_pool(name="ps3", bufs=4, space="PSUM"))

            ln2w_bc = ln2b_bc = None
            if apply_ln2_affine:
                cst3 = pf.enter_context(tc.tile_pool(name="cst3", bufs=1))
                ln2w_bc = cst3.tile([128, D], F32, tag="ln2wbc")
                ln2b_bc = cst3.tile([128, D], F32, tag="ln2bbc")
                lw_f = cst3.tile([1, D], F32, tag="lnwf")
                nc.sync.dma_start(out=lw_f, in_=ln2w_in[:, :])
                lb_f = cst3.tile([1, D], F32, tag="lnbf")
                nc.sync.dma_start(out=lb_f, in_=ln2b_in[:, :])
                wr_ = cst3.tile([1, D], F32R, tag="lnwr")
                nc.vector.tensor_copy(wr_, lw_f)
                br_ = cst3.tile([1, D], F32R, tag="lnbr")
                nc.vector.tensor_copy(br_, lb_f)
                for nn in range(2):
                    bps = ps3.tile([128, 512], F32, tag="p3")
                    nc.tensor.matmul(bps, ones_row,
                                     wr_[:, 512 * nn:512 * (nn + 1)],
                                     start=True, stop=True)
                    nc.scalar.copy(ln2w_bc[:, 512 * nn:512 * (nn + 1)], bps)
                    bps2 = ps3.tile([128, 512], F32, tag="p3")
                    nc.tensor.matmul(bps2, ones_row,
                                     br_[:, 512 * nn:512 * (nn + 1)],
                                     start=True, stop=True)
                    nc.scalar.copy(ln2b_bc[:, 512 * nn:512 * (nn + 1)], bps2)

            for b in range(nbatch if "3" in phases else 0):
                z2t_ = z2tp.tile([128, KC, S], F32, tag="z2t")
                nc.sync.dma_start(
                    out=z2t_, in_=z2_scr[b].rearrange("kc p s -> p kc s"))
                for sc in range(SC):
                    nat = natp.tile([128, D], F32, tag="natf")
                    for half in range(2):
                        np_ = ps3.tile([128, 512], F32, tag="p3")
                        for dq in range(4):
                            dcc = 4 * half + dq
                            nc.tensor.transpose(
                                np_[:, 128 * dq:128 * (dq + 1)],
                                z2t_[:, dcc, 128 * sc:128 * (sc + 1)], ident)
                        nc.scalar.copy(nat[:, 512 * half:512 * (half + 1)], np_)
                    stats = st3.tile([128, 2, 6], F32, tag="bns")
                    for g in range(2):
                        nc.vector.bn_stats(out=stats[:, g, :],
                                           in_=nat[:, 512 * g:512 * (g + 1)])
                    mv = st3.tile([128, 2], F32, tag="bnagg")
                    nc.vector.bn_aggr(out=mv, in_=stats)
                    lvar = st3.tile([128, 1], F32, tag="lv3")
                    nc.scalar.activation(lvar, mv[:, 1:2], AF.Ln, bias=eps_128)
                    rstd = st3.tile([128, 1], F32, tag="rs3")
                    nc.scalar.activation(rstd, lvar, AF.Exp, scale=-0.5)
                    xo = natp.tile([128, D], F32, tag="xo")
                    nc.vector.tensor_scalar(
                        out=xo, in0=nat, scalar1=mv[:, 0:1], scalar2=rstd,
                        op0=OP.subtract, op1=OP.mult)
                    if apply_ln2_affine:
                        xo2 = natp.tile([128, D], F32, tag="xo2")
                        nc.vector.tensor_mul(xo2, xo, ln2w_bc)
                        nc.vector.tensor_add(xo2, xo2, ln2b_bc)
                        xo = xo2
                    nc.sync.dma_start(
                        out=x_out[b, 128 * sc:128 * (sc + 1), :], in_=xo)
    return nc


_cache = {}


def kernel(**inputs):
    inp = {k_: np.asarray(v) for k_, v in inputs.items()}
    mask = int(inp["mask"])
    gammas = inp["gammas"].astype(np.float64).reshape(H)
    gam_neg = [-float(np.log1p(np.exp(g))) for g in gammas]

    ln1w, ln1b = inp["ln1_w"], inp["ln1_b"]
    ln2w, ln2b = inp["ln2_w"], inp["ln2_b"]
    apply_ln1 = not (np.all(ln1w == 1.0) and np.all(ln1b == 0.0))
    apply_ln2 = not (np.all(ln2w == 1.0) and np.all(ln2b == 0.0))

    idx = np.arange(S)
    allow_s = ((idx[None, :] - idx[:, None]) < mask).astype(np.float16)
    pos_neg = np.abs(idx[None, :] - idx[:, None]).astype(np.float16)

    cols = lambda v, n: np.ascontiguousarray(
        np.asarray(v).astype(np.float32).reshape(n, 128).T)
    W1p = np.ascontiguousarray(
        inp["W1"].astype(np.float32).reshape(KC, 128, FC, 128)
        .transpose(2, 1, 0, 3).reshape(FC, 128, D))

    common = {
        "Wk": np.ascontiguousarray(inp["Wk"].astype(np.float32)),
        "Wv": np.ascontiguousarray(inp["Wv"].astype(np.float32)),
        "Wo": np.ascontiguousarray(inp["Wo"].astype(np.float32)),
        "W1p": W1p,
        "W2": np.ascontiguousarray(inp["W2"].astype(np.float32)),
        "bkc": cols(inp["bk"], KC), "boc": cols(inp["bo"], KC),
        "b1c": cols(inp["b1"], FC), "b2c": cols(inp["b2"], KC),
        "bvr": inp["bv"].astype(np.float32).reshape(1, D),
        "allow_s": allow_s, "pos_neg": pos_neg,
        "ln1wc": cols(ln1w, KC), "ln1bc": cols(ln1b, KC),
        "ln2wb": ln2w.astype(np.float32).reshape(1, D),
        "ln2bb": ln2b.astype(np.float32).reshape(1, D),
    }

    apply_bv = bool(np.any(inp["bv"] != 0.0))
    apply_bk = bool(np.any(inp["bk"] != 0.0))
    key = (mask, tuple(gam_neg), apply_ln1, apply_ln2, apply_bv, apply_bk)
    if key not in _cache:
        nc = build_nc(mask, gam_neg, apply_ln1, apply_ln2,
                      apply_bv=apply_bv, apply_bk=apply_bk)
        nc.finalize()
        _cache[key] = nc
    nc = _cache[key]

    q = np.ascontiguousarray(inp["query"].astype(np.float32))
    k = np.ascontiguousarray(inp["keys"].astype(np.float32))
    v = np.ascontiguousarray(inp["values"].astype(np.float32))
    in_maps = []
    for c in range(NC_):
        sl = slice(BSH * c, BSH * (c + 1))
        m = dict(common)
        m["q"], m["k"], m["v"] = q[sl], k[sl], v[sl]
        in_maps.append(m)

    res = run_bass_kernel_spmd(nc, in_maps, list(range(NC_)))
    x = np.concatenate([r["x_out"] for r in res.results], axis=0)
    p = np.concatenate([r["p_out"] for r in res.results], axis=0)
    return x, p
